# revision 24
# baseline (speedup 1.0000x reference)
"""Depthwise-separable conv2d block (dw3x3 + BN + ReLU + map-cut, pw1x1 + BN +
ReLU + map-cut) on 8 Trainium2 NeuronCores, data-parallel over the batch dim.

Fixed problem shapes: x (32,256,56,56) f32 -> out (32,512,54,54) f32.

v3: fp8 DoubleRow depthwise + single-pass pointwise epilogue + dw/pw
interleave.
  - depthwise 3x3 VALID conv in fp8e4m3 with perf_mode=DoubleRow: the two
    contraction "k-tiles" of each matmul are two taps of the same channel
    (the second rhs k-tile is the same x window shifted by the inter-tap
    offset), so 9 taps cost 4 double-pumped matmuls + 1 plain one per
    486-column chunk; f32 PSUM accumulation, per-chunk fused relu+bias ACT
    drain, DVE takes the per-chunk max off PSUM for the dw cut mask
  - the per-(image,channel) depthwise cut mask is folded into per-image
    masked copies of the pointwise weights (w2 * mask), computed on ACT
  - pointwise 1x1 conv in bf16 (images 0..2): K=2x128 GEMM into two 3-bank
    PSUM half-groups; the map stays resident in PSUM, DVE reduces the map
    max off PSUM, and one ACT pass per half applies relu((z+b2)*mask)
    while draining PSUM -> SBUF; the next image's dw chunks are emitted
    between pw m-tiles so the tensor engine streams matmuls while the
    drains wait on the mask
  - tail m-tiles (no dw filler left): drains run immediately with bias
    b2-thresh and an accum (sum) side output; since
    max(relu(z+b2)) >= th  <=>  sum(relu(z+b2-th)) > 0, the cut mask
    comes from the accums and DVE applies it as a cheap multiply
BatchNorm (inference) is folded into the conv weights/biases on the host.
"""

import ml_dtypes
import numpy as np

import concourse.bacc as bacc
import concourse.bass as bass
import concourse.mybir as mybir
import concourse.tile as tile
from concourse.bass_utils import run_bass_kernel_spmd

EPS = 1e-5
DW_THRESH = 4.0
PW_THRESH = 0.001

B, CIN, COUT, H, W = 32, 256, 512, 56, 56
HO, WO = 54, 54
NPIX = HO * WO          # 2916
NCORES = 8
BPC = B // NCORES       # 4 images per core
P = 128                 # partitions
KT = CIN // P           # 2 cin tiles
MT = COUT // P          # 4 cout tiles
NCH = 6                 # output chunks per map: 6 x (9 rows x 54 cols)
CHROWS = HO // NCH      # 9
CHUNK = CHROWS * WO     # 486 columns per chunk (one PSUM bank)
HCH = NCH // 2          # 3 chunks per pw PSUM half-group
BANK = 512              # fp32 elems per PSUM bank
HPIX = HCH * CHUNK      # 1458
QPIX = NPIX // 4        # 729 (quarter stores on the last m-tile)

USE_DVE_STT = True

DW_FP8 = True
# tap pairs for DoubleRow: (base_di, base_dj, delta); delta is the flat
# x offset between the pair's two taps (w==0 for the pad tap of pair 4)
DW_PAIRS = [(0, 0, 1), (0, 2, 54), (1, 1, 1), (2, 0, 1), (2, 2, 0)]

F32 = mybir.dt.float32
BF16 = mybir.dt.bfloat16
FP8 = mybir.dt.float8e4
RELU = mybir.ActivationFunctionType.Relu
COPY = mybir.ActivationFunctionType.Copy
MAX = mybir.AluOpType.max
ADD = mybir.AluOpType.add
MULT = mybir.AluOpType.mult

_cached_nc = None


def _build_program():
    nc = bacc.Bacc("TRN2", target_bir_lowering=False, debug=False)

    xdt = FP8 if DW_FP8 else BF16
    xs = nc.dram_tensor("xs", [BPC, CIN, H, W], xdt, kind="ExternalInput").ap()
    if DW_FP8:
        dwdiag = nc.dram_tensor(
            "dwdiag", [P, KT, 5, 2, P], FP8, kind="ExternalInput").ap()
    else:
        dwdiag = nc.dram_tensor(
            "dwdiag", [P, KT, 9, P], BF16, kind="ExternalInput").ap()
    w2t = nc.dram_tensor("w2t", [P, KT, COUT], BF16, kind="ExternalInput").ap()
    b1t = nc.dram_tensor("b1t", [P, KT], F32, kind="ExternalInput").ap()
    b2t = nc.dram_tensor("b2t", [P, MT], F32, kind="ExternalInput").ap()
    zs = nc.dram_tensor("zs", [BPC, COUT, HO, WO], F32, kind="ExternalOutput").ap()
    zs_flat = zs.rearrange("b c h w -> b c (h w)")

    with tile.TileContext(nc) as tc:
        with (
            tc.tile_pool(name="consts", bufs=1) as consts,
            tc.tile_pool(name="xp", bufs=8) as xp,
            tc.tile_pool(name="yp", bufs=6) as yp,
            tc.tile_pool(name="zp", bufs=4) as zp,
            tc.tile_pool(name="wm", bufs=6) as wmp,
            tc.tile_pool(name="st", bufs=40) as st,
            tc.tile_pool(name="psdw", bufs=2, space="PSUM") as psdw,
            tc.tile_pool(name="pspw", bufs=2, space="PSUM") as pspw,
        ):
            xtiles = {}

            def load_x(b, k, slabs=1):
                X = xp.tile([P, H, W], xdt, name="X")
                if slabs == 1:
                    nc.gpsimd.dma_start(out=X, in_=xs[b, k * P:(k + 1) * P, :, :])
                else:
                    # row-sliced load so the first chunks can start early
                    bounds = [0, 15, 29, 43, H]
                    for s in range(4):
                        r0, r1 = bounds[s], bounds[s + 1]
                        nc.gpsimd.dma_start(
                            out=X[:, r0:r1, :],
                            in_=xs[b, k * P:(k + 1) * P, r0:r1, :])
                xtiles[b, k] = X

            # weight/bias loads on the (otherwise idle at boot) scalar and
            # sync engines; x loads on gpsimd - triggers issue in parallel.
            # k=1 runs first, so its weights and x tile load first.
            if DW_FP8:
                dwsb = consts.tile([P, KT, 5, 2, P], FP8)
            else:
                dwsb = consts.tile([P, KT, 9, P], BF16)
            nc.sync.dma_start(out=dwsb[:, 1], in_=dwdiag[:, 1])
            b1sb = consts.tile([P, KT], F32)
            nc.sync.dma_start(out=b1sb, in_=b1t)
            load_x(0, 1, slabs=4)
            load_x(0, 0)
            nc.sync.dma_start(out=dwsb[:, 0], in_=dwdiag[:, 0])
            w2sb = consts.tile([P, KT, COUT], BF16)
            nc.scalar.dma_start(out=w2sb, in_=w2t)
            b2sb = consts.tile([P, MT], F32)
            nc.scalar.dma_start(out=b2sb, in_=b2t)
            for b in range(1, BPC):
                for k in range(KT):
                    load_x(b, k)
            # b2 - thresh, for the deferred (sum-trick) epilogue
            b2thsb = consts.tile([P, MT], F32)
            nc.vector.tensor_scalar(
                out=b2thsb, in0=b2sb, scalar1=PW_THRESH, scalar2=None,
                op0=mybir.AluOpType.subtract)

            # zero column for stt-style relu on DVE/GpSimd
            zerosb = consts.tile([P, 1], F32)
            nc.vector.memset(zerosb, 0.0)

            ytiles = {}         # (b, k) -> Y [P, NCH, CHUNK] bf16 view
            mcxtiles = {}       # (b, k) -> [P, NCH] f32 per-chunk conv maxes
            wmtiles = {}        # (b, k) -> [P, COUT] bf16 masked pw weights
            KS = [1, 0]         # k1 first: its cut mask is ready earliest

            def emit_dw_chunk(b, k, n):
                """Accumulating matmuls, fused relu+bias ACT drain, and a
                DVE per-chunk max off PSUM (for the dw cut mask)."""
                if (b, k) not in ytiles:
                    Y = yp.tile([P, NPIX], BF16, name="Y")
                    ytiles[b, k] = Y.rearrange("p (c x) -> p c x", x=CHUNK)
                    mcxtiles[b, k] = st.tile([P, NCH], F32, name="mcx")
                X = xtiles[b, k]
                P1 = psdw.tile([P, BANK], F32, name="P1")
                r0 = CHROWS * n
                if DW_FP8:
                    for p, (di, dj, delta) in enumerate(DW_PAIRS):
                        V = X[:, r0 + di: r0 + di + CHROWS, dj: dj + WO]
                        rhs = bass.AP(
                            tensor=V.tensor, offset=V.offset,
                            ap=[V.ap[0], [delta, 2], V.ap[1], V.ap[2]])
                        nc.tensor.matmul(
                            P1[:, 0:CHUNK],
                            lhsT=dwsb[:, k, p, :, :],
                            rhs=rhs,
                            start=(p == 0),
                            stop=(p == len(DW_PAIRS) - 1),
                            perf_mode=mybir.MatmulPerfMode.DoubleRow,
                        )
                else:
                    for t in range(9):
                        di, dj = t // 3, t % 3
                        rhs = X[:, r0 + di: r0 + di + CHROWS, dj: dj + WO]
                        nc.tensor.matmul(
                            P1[:, 0:CHUNK],
                            lhsT=dwsb[:, k, t, :],
                            rhs=rhs,
                            start=(t == 0),
                            stop=(t == 8),
                        )
                nc.vector.tensor_reduce(
                    mcxtiles[b, k][:, n:n + 1], P1[:, 0:CHUNK],
                    axis=mybir.AxisListType.X, op=MAX)
                nc.scalar.activation(
                    out=ytiles[b, k][:, n, :],
                    in_=P1[:, 0:CHUNK],
                    func=RELU,
                    bias=b1sb[:, k:k + 1], scale=1.0)

            def emit_dw_mask(b, k):
                """mask1 = (max(conv) + b1 >= 4), w2m = w2 * mask1 (on ACT)."""
                m1 = st.tile([P, 1], F32, name="m1")
                nc.vector.tensor_reduce(
                    m1, mcxtiles[b, k], axis=mybir.AxisListType.X, op=MAX)
                mask1 = st.tile([P, 1], F32, name="mask1")
                nc.vector.tensor_scalar(
                    out=mask1, in0=m1, scalar1=b1sb[:, k:k + 1],
                    scalar2=DW_THRESH, op0=ADD, op1=mybir.AluOpType.is_ge)
                w2m = wmp.tile([P, COUT], BF16, name="w2m")
                nc.scalar.activation(
                    out=w2m, in_=w2sb[:, k, :], func=COPY, bias=0.0,
                    scale=mask1)
                wmtiles[b, k] = w2m

            def emit_pw_half_mms(PZ, b, m, half):
                for j in range(HCH):
                    n = half * HCH + j
                    for i, k in enumerate(KS):
                        nc.tensor.matmul(
                            PZ[:, j, 0:CHUNK],
                            lhsT=wmtiles[b, k][:, m * P:(m + 1) * P],
                            rhs=ytiles[b, k][:, n, :],
                            start=(i == 0),
                            stop=(i == KT - 1),
                        )

            # deferred per-engine closures so each engine's emission order
            # matches its runtime-ready order (engines execute in order)
            pend = {"dve_tail": [], "actA": [], "actB": []}

            def pop(q):
                if pend[q]:
                    pend[q].pop(0)()

            def emit_pw_fused(b, m):
                """12 matmuls into two 3-bank PSUM half-groups, then the
                first half's map max; queue the rest of the epilogue."""
                PZ = [pspw.tile([P, HCH, BANK], F32, name="PZ"),
                      pspw.tile([P, HCH, BANK], F32, name="PZ")]
                emit_pw_half_mms(PZ[0], b, m, 0)
                emit_pw_half_mms(PZ[1], b, m, 1)

                mab = st.tile([P, 2], F32, name="mab")
                # half-A map max can run as soon as the A matmuls finish
                nc.vector.tensor_reduce(
                    mab[:, 0:1], PZ[0][:, :, 0:CHUNK],
                    axis=mybir.AxisListType.XY, op=MAX)

                mask2 = st.tile([P, 1], F32, name="mask2")
                b2m = st.tile([P, 1], F32, name="b2m")

                def dve_tail():
                    nc.vector.tensor_reduce(
                        mab[:, 1:2], PZ[1][:, :, 0:CHUNK],
                        axis=mybir.AxisListType.XY, op=MAX)
                    m2 = st.tile([P, 1], F32, name="m2")
                    nc.vector.tensor_tensor(
                        m2, mab[:, 0:1], mab[:, 1:2], op=MAX)
                    # mask2 = (max(z) + b2 >= th)
                    nc.vector.tensor_scalar(
                        out=mask2, in0=m2, scalar1=b2sb[:, m:m + 1],
                        scalar2=PW_THRESH, op0=ADD, op1=mybir.AluOpType.is_ge)
                    nc.vector.tensor_tensor(
                        b2m, b2sb[:, m:m + 1], mask2, op=MULT)
                pend["dve_tail"].append(dve_tail)

                Z = zp.tile([P, NPIX], F32, name="Z")
                Z3 = Z.rearrange("p (c x) -> p c x", x=CHUNK)

                def act_drain(half):
                    # z = relu((z_nb + b2) * mask): scale=mask, bias=b2*mask
                    nc.scalar.activation(
                        out=Z3[:, half * HCH:(half + 1) * HCH, :],
                        in_=PZ[half][:, :, 0:CHUNK],
                        func=RELU, bias=b2m, scale=mask2)
                    nc.sync.dma_start(
                        out=zs_flat[b, m * P:(m + 1) * P,
                                    half * HPIX:(half + 1) * HPIX],
                        in_=Z[:, half * HPIX:(half + 1) * HPIX])
                pend["actA"].append(lambda: act_drain(0))
                pend["actB"].append(lambda: act_drain(1))

            def emit_pw_deferred(b, m, last=False):
                """No-filler m-tiles: drains run immediately (bias b2 - th)
                with accum side-sums; mask = (sum > 0), applied as a
                multiply. Work spread over ACT (drain A), DVE (drain B,
                mask, mult B) and GpSimd (mult A)."""
                PZ = [pspw.tile([P, HCH, BANK], F32, name="PZ"),
                      pspw.tile([P, HCH, BANK], F32, name="PZ")]
                Z = zp.tile([P, NPIX], F32, name="Z")
                Z3 = Z.rearrange("p (c x) -> p c x", x=CHUNK)
                asum = st.tile([P, 2], F32, name="asum")
                zeros3 = zerosb[:, 0:1].to_broadcast([P, HCH, CHUNK])
                emit_pw_half_mms(PZ[0], b, m, 0)
                nc.scalar.activation(
                    out=Z3[:, 0:HCH, :], in_=PZ[0][:, :, 0:CHUNK],
                    func=RELU, bias=b2thsb[:, m:m + 1], scale=1.0,
                    accum_out=asum[:, 0:1])
                emit_pw_half_mms(PZ[1], b, m, 1)
                nc.scalar.activation(
                    out=Z3[:, HCH:NCH, :], in_=PZ[1][:, :, 0:CHUNK],
                    func=RELU, bias=b2thsb[:, m:m + 1], scale=1.0,
                    accum_out=asum[:, 1:2])
                mask2 = st.tile([P, 1], F32, name="mask2")
                nc.vector.tensor_tensor(
                    mask2, asum[:, 0:1], asum[:, 1:2], op=ADD)
                nc.vector.tensor_scalar(
                    out=mask2, in0=mask2, scalar1=0.0, scalar2=None,
                    op0=mybir.AluOpType.is_gt)

                def mult_store(lo, hi, dma):
                    zpart = Z[:, lo:hi]
                    nc.vector.tensor_scalar(
                        out=zpart, in0=zpart, scalar1=mask2,
                        scalar2=None, op0=MULT)
                    dma(out=zs_flat[b, m * P:(m + 1) * P, lo:hi], in_=zpart)

                if last:
                    # quarter-granular masking and stores to shorten the
                    # final drain-out (gpsimd triggers avoided: its end
                    # drain would become the finish-barrier long pole)
                    mult_store(0, QPIX, nc.sync.dma_start)
                    mult_store(QPIX, 2 * QPIX, nc.scalar.dma_start)
                    mult_store(2 * QPIX, 3 * QPIX, nc.sync.dma_start)
                    mult_store(3 * QPIX, NPIX, nc.scalar.dma_start)
                else:
                    mult_store(0, HPIX, nc.sync.dma_start)
                    mult_store(HPIX, NPIX, nc.sync.dma_start)

            # ------------- program -------------
            # image 0 depthwise, solid (k1 first so its mask is ready when
            # the first pw m-tile starts)
            for k in KS:
                for n in range(NCH):
                    emit_dw_chunk(0, k, n)
                emit_dw_mask(0, k)

            # steady state: pw(b) m-tiles with dw(b+1) chunks as filler.
            # per-group emission order is tuned so each engine's in-order
            # queue matches readiness: [maxB+mask chain (m-1)] [fill f0]
            # [drain A(m-1)] [drain B(m-1)] [fill f1] [fill f2 (+ dw mask)]
            # [pw mms (m)] [maxA(m)]
            FILLS = [3, 3, 3, 3]
            for b in range(BPC - 1):
                chunks = [(k, n) for k in KS for n in range(NCH)]
                ci = 0
                for m in range(MT):
                    pop("dve_tail")
                    for j in range(FILLS[m]):
                        k, n = chunks[ci]
                        ci += 1
                        emit_dw_chunk(b + 1, k, n)
                        if j == 0:
                            pop("actA")
                        if j == 1:
                            pop("actB")
                        if n == NCH - 1:
                            emit_dw_mask(b + 1, k)
                    if b == BPC - 2 and m == MT - 1:
                        # last fused slot: go deferred so the b3 phase is
                        # not gated on this m-tile's mask chain
                        pop("dve_tail")
                        pop("actA")
                        pop("actB")
                        emit_pw_deferred(b, m)
                    else:
                        emit_pw_fused(b, m)
            # last image: deferred-mask epilogue throughout
            for m in range(MT):
                emit_pw_deferred(BPC - 1, m, last=(m == MT - 1))
    nc.compile()
    return nc


def _prep_params(dw_w, dw_b, dw_gamma, dw_beta, dw_mean, dw_var,
                 pw_w, pw_b, pw_gamma, pw_beta, pw_mean, pw_var):
    dw_scale = dw_gamma / np.sqrt(dw_var + EPS)
    b1 = dw_b * dw_scale + dw_beta - dw_mean * dw_scale          # (256,)
    w1 = dw_w[:, 0] * dw_scale[:, None, None]                    # (256,3,3)

    idx = np.arange(P)
    if DW_FP8:
        dwdiag = np.zeros((P, KT, 5, 2, P), np.float32)
        for k in range(KT):
            for p, (di, dj, _) in enumerate(DW_PAIRS):
                taps = [(di, dj)]
                flat = di * 3 + dj
                if flat + 1 < 9:
                    taps.append(((flat + 1) // 3, (flat + 1) % 3))
                for j, (ti, tj) in enumerate(taps):
                    dwdiag[idx, k, p, j, idx] = w1[k * P:(k + 1) * P, ti, tj]
        dwdiag = np.ascontiguousarray(dwdiag).astype(ml_dtypes.float8_e4m3)
    else:
        dwdiag = np.zeros((P, KT, 9, P), np.float32)
        for k in range(KT):
            for t in range(9):
                dwdiag[idx, k, t, idx] = w1[k * P:(k + 1) * P, t // 3, t % 3]
        dwdiag = np.ascontiguousarray(dwdiag).astype(ml_dtypes.bfloat16)

    pw_scale = pw_gamma / np.sqrt(pw_var + EPS)
    b2 = pw_b * pw_scale + pw_beta - pw_mean * pw_scale          # (512,)
    w2 = pw_w * pw_scale[:, None]                                # (512,256)
    # w2t[ck, k, o] = w2[o, k*128+ck]
    w2t = np.ascontiguousarray(
        w2.T.reshape(KT, P, COUT).transpose(1, 0, 2)).astype(np.float32)
    b1t = np.ascontiguousarray(b1.reshape(KT, P).T).astype(np.float32)
    b2t = np.ascontiguousarray(b2.reshape(MT, P).T).astype(np.float32)
    return (dwdiag, w2t.astype(ml_dtypes.bfloat16), b1t, b2t)


def kernel(x, dw_w, dw_b, dw_gamma, dw_beta, dw_mean, dw_var,
           pw_w, pw_b, pw_gamma, pw_beta, pw_mean, pw_var):
    global _cached_nc
    x = np.ascontiguousarray(np.asarray(x, np.float32))
    args = [np.asarray(a, np.float32) for a in
            (dw_w, dw_b, dw_gamma, dw_beta, dw_mean, dw_var,
             pw_w, pw_b, pw_gamma, pw_beta, pw_mean, pw_var)]
    dwdiag, w2t, b1t, b2t = _prep_params(*args)
    xdt = ml_dtypes.float8_e4m3 if DW_FP8 else ml_dtypes.bfloat16
    x16 = x.astype(xdt)

    if _cached_nc is None:
        _cached_nc = _build_program()
    nc = _cached_nc

    in_maps = []
    for c in range(NCORES):
        in_maps.append({
            "xs": np.ascontiguousarray(x16[c * BPC:(c + 1) * BPC]),
            "dwdiag": dwdiag,
            "w2t": w2t,
            "b1t": b1t,
            "b2t": b2t,
        })
    res = run_bass_kernel_spmd(nc, in_maps, core_ids=list(range(NCORES)))
    out = np.concatenate([res.results[c]["zs"] for c in range(NCORES)], axis=0)
    return out


# revision 25
# speedup vs baseline: 1.0185x; 1.0185x over previous
"""Depthwise-separable conv2d block (dw3x3 + BN + ReLU + map-cut, pw1x1 + BN +
ReLU + map-cut) on 8 Trainium2 NeuronCores, data-parallel over the batch dim.

Fixed problem shapes: x (32,256,56,56) f32 -> out (32,512,54,54) f32.

v3: fp8 DoubleRow depthwise + single-pass pointwise epilogue + dw/pw
interleave.
  - depthwise 3x3 VALID conv in fp8e4m3 with perf_mode=DoubleRow: the two
    contraction "k-tiles" of each matmul are two taps of the same channel
    (the second rhs k-tile is the same x window shifted by the inter-tap
    offset), so 9 taps cost 4 double-pumped matmuls + 1 plain one per
    486-column chunk; f32 PSUM accumulation, per-chunk fused relu+bias ACT
    drain, DVE takes the per-chunk max off PSUM for the dw cut mask
  - the per-(image,channel) depthwise cut mask is folded into per-image
    masked copies of the pointwise weights (w2 * mask), computed on ACT
  - pointwise 1x1 conv in bf16 (images 0..2): K=2x128 GEMM into two 3-bank
    PSUM half-groups; the map stays resident in PSUM, DVE reduces the map
    max off PSUM, and one ACT pass per half applies relu((z+b2)*mask)
    while draining PSUM -> SBUF; the next image's dw chunks are emitted
    between pw m-tiles so the tensor engine streams matmuls while the
    drains wait on the mask
  - tail m-tiles (no dw filler left): drains run immediately with bias
    b2-thresh and an accum (sum) side output; since
    max(relu(z+b2)) >= th  <=>  sum(relu(z+b2-th)) > 0, the cut mask
    comes from the accums and DVE applies it as a cheap multiply
BatchNorm (inference) is folded into the conv weights/biases on the host.
"""

import ml_dtypes
import numpy as np

import concourse.bacc as bacc
import concourse.bass as bass
import concourse.mybir as mybir
import concourse.tile as tile
from concourse.bass_utils import run_bass_kernel_spmd

EPS = 1e-5
DW_THRESH = 4.0
PW_THRESH = 0.001

B, CIN, COUT, H, W = 32, 256, 512, 56, 56
HO, WO = 54, 54
NPIX = HO * WO          # 2916
NCORES = 8
BPC = B // NCORES       # 4 images per core
P = 128                 # partitions
KT = CIN // P           # 2 cin tiles
MT = COUT // P          # 4 cout tiles
NCH = 6                 # output chunks per map: 6 x (9 rows x 54 cols)
CHROWS = HO // NCH      # 9
CHUNK = CHROWS * WO     # 486 columns per chunk (one PSUM bank)
HCH = NCH // 2          # 3 chunks per pw PSUM half-group
BANK = 512              # fp32 elems per PSUM bank
HPIX = HCH * CHUNK      # 1458
QPIX = NPIX // 4        # 729 (quarter stores on the last m-tile)

USE_DVE_STT = True

DW_FP8 = True
# tap pairs for DoubleRow: (base_di, base_dj, delta); delta is the flat
# x offset between the pair's two taps (w==0 for the pad tap of pair 4)
DW_PAIRS = [(0, 0, 1), (0, 2, 54), (1, 1, 1), (2, 0, 1), (2, 2, 0)]

F32 = mybir.dt.float32
BF16 = mybir.dt.bfloat16
FP8 = mybir.dt.float8e4
RELU = mybir.ActivationFunctionType.Relu
COPY = mybir.ActivationFunctionType.Copy
MAX = mybir.AluOpType.max
ADD = mybir.AluOpType.add
MULT = mybir.AluOpType.mult

_cached_nc = None


def _build_program():
    nc = bacc.Bacc("TRN2", target_bir_lowering=False, debug=False)

    xdt = FP8 if DW_FP8 else BF16
    xs = nc.dram_tensor("xs", [BPC, CIN, H, W], xdt, kind="ExternalInput").ap()
    if DW_FP8:
        dwdiag = nc.dram_tensor(
            "dwdiag", [P, KT, 5, 2, P], FP8, kind="ExternalInput").ap()
    else:
        dwdiag = nc.dram_tensor(
            "dwdiag", [P, KT, 9, P], BF16, kind="ExternalInput").ap()
    w2t = nc.dram_tensor("w2t", [P, KT, COUT], BF16, kind="ExternalInput").ap()
    b1t = nc.dram_tensor("b1t", [P, KT], F32, kind="ExternalInput").ap()
    b2t = nc.dram_tensor("b2t", [P, MT], F32, kind="ExternalInput").ap()
    zs = nc.dram_tensor("zs", [BPC, COUT, HO, WO], F32, kind="ExternalOutput").ap()
    zs_flat = zs.rearrange("b c h w -> b c (h w)")

    with tile.TileContext(nc) as tc:
        with (
            tc.tile_pool(name="consts", bufs=1) as consts,
            tc.tile_pool(name="xp", bufs=8) as xp,
            tc.tile_pool(name="yp", bufs=4) as yp,
            tc.tile_pool(name="zp", bufs=3) as zp,
            tc.tile_pool(name="wm", bufs=4) as wmp,
            tc.tile_pool(name="st", bufs=32) as st,
            tc.tile_pool(name="psdw", bufs=2, space="PSUM") as psdw,
            tc.tile_pool(name="pspw", bufs=2, space="PSUM") as pspw,
        ):
            xtiles = {}

            def load_x(b, k, slabs=1):
                X = xp.tile([P, H, W], xdt, name="X")
                if slabs == 1:
                    nc.gpsimd.dma_start(out=X, in_=xs[b, k * P:(k + 1) * P, :, :])
                else:
                    # row-sliced load so the first chunks can start early
                    bounds = [0, 15, 29, 43, H]
                    for s in range(4):
                        r0, r1 = bounds[s], bounds[s + 1]
                        nc.gpsimd.dma_start(
                            out=X[:, r0:r1, :],
                            in_=xs[b, k * P:(k + 1) * P, r0:r1, :])
                xtiles[b, k] = X

            # weight/bias loads on the (otherwise idle at boot) scalar and
            # sync engines; x loads on gpsimd - triggers issue in parallel.
            # k=1 runs first, so its weights and x tile load first.
            if DW_FP8:
                dwsb = consts.tile([P, KT, 5, 2, P], FP8)
            else:
                dwsb = consts.tile([P, KT, 9, P], BF16)
            nc.sync.dma_start(out=dwsb[:, 1], in_=dwdiag[:, 1])
            b1sb = consts.tile([P, KT], F32)
            nc.sync.dma_start(out=b1sb, in_=b1t)
            load_x(0, 1, slabs=4)
            load_x(0, 0)
            nc.sync.dma_start(out=dwsb[:, 0], in_=dwdiag[:, 0])
            w2sb = consts.tile([P, KT, COUT], BF16)
            nc.scalar.dma_start(out=w2sb, in_=w2t)
            b2sb = consts.tile([P, MT], F32)
            nc.scalar.dma_start(out=b2sb, in_=b2t)
            for b in range(1, BPC):
                for k in range(KT):
                    load_x(b, k)
            # b2 - thresh, for the deferred (sum-trick) epilogue
            b2thsb = consts.tile([P, MT], F32)
            nc.vector.tensor_scalar(
                out=b2thsb, in0=b2sb, scalar1=PW_THRESH, scalar2=None,
                op0=mybir.AluOpType.subtract)

            # zero column for stt-style relu on DVE/GpSimd
            zerosb = consts.tile([P, 1], F32)
            nc.vector.memset(zerosb, 0.0)

            ytiles = {}         # (b, k) -> Y [P, NCH, CHUNK] bf16 view
            mcxtiles = {}       # (b, k) -> [P, NCH] f32 per-chunk conv maxes
            wmtiles = {}        # (b, k) -> [P, COUT] bf16 masked pw weights
            KS = [1, 0]         # k1 first: its cut mask is ready earliest

            def emit_dw_chunk(b, k, n):
                """Accumulating matmuls, fused relu+bias ACT drain, and a
                DVE per-chunk max off PSUM (for the dw cut mask)."""
                if (b, k) not in ytiles:
                    Y = yp.tile([P, NPIX], BF16, name="Y")
                    ytiles[b, k] = Y.rearrange("p (c x) -> p c x", x=CHUNK)
                    mcxtiles[b, k] = st.tile([P, NCH], F32, name="mcx")
                X = xtiles[b, k]
                P1 = psdw.tile([P, BANK], F32, name="P1")
                r0 = CHROWS * n
                if DW_FP8:
                    for p, (di, dj, delta) in enumerate(DW_PAIRS):
                        V = X[:, r0 + di: r0 + di + CHROWS, dj: dj + WO]
                        rhs = bass.AP(
                            tensor=V.tensor, offset=V.offset,
                            ap=[V.ap[0], [delta, 2], V.ap[1], V.ap[2]])
                        nc.tensor.matmul(
                            P1[:, 0:CHUNK],
                            lhsT=dwsb[:, k, p, :, :],
                            rhs=rhs,
                            start=(p == 0),
                            stop=(p == len(DW_PAIRS) - 1),
                            perf_mode=mybir.MatmulPerfMode.DoubleRow,
                        )
                else:
                    for t in range(9):
                        di, dj = t // 3, t % 3
                        rhs = X[:, r0 + di: r0 + di + CHROWS, dj: dj + WO]
                        nc.tensor.matmul(
                            P1[:, 0:CHUNK],
                            lhsT=dwsb[:, k, t, :],
                            rhs=rhs,
                            start=(t == 0),
                            stop=(t == 8),
                        )
                nc.vector.tensor_reduce(
                    mcxtiles[b, k][:, n:n + 1], P1[:, 0:CHUNK],
                    axis=mybir.AxisListType.X, op=MAX)
                nc.scalar.activation(
                    out=ytiles[b, k][:, n, :],
                    in_=P1[:, 0:CHUNK],
                    func=RELU,
                    bias=b1sb[:, k:k + 1], scale=1.0)

            def emit_dw_mask(b, k):
                """mask1 = (max(conv) + b1 >= 4), w2m = w2 * mask1 (on ACT)."""
                m1 = st.tile([P, 1], F32, name="m1")
                nc.vector.tensor_reduce(
                    m1, mcxtiles[b, k], axis=mybir.AxisListType.X, op=MAX)
                mask1 = st.tile([P, 1], F32, name="mask1")
                nc.vector.tensor_scalar(
                    out=mask1, in0=m1, scalar1=b1sb[:, k:k + 1],
                    scalar2=DW_THRESH, op0=ADD, op1=mybir.AluOpType.is_ge)
                w2m = wmp.tile([P, COUT], BF16, name="w2m")
                nc.scalar.activation(
                    out=w2m, in_=w2sb[:, k, :], func=COPY, bias=0.0,
                    scale=mask1)
                wmtiles[b, k] = w2m

            def emit_pw_half_mms(PZ, b, m, half):
                for j in range(HCH):
                    n = half * HCH + j
                    for i, k in enumerate(KS):
                        nc.tensor.matmul(
                            PZ[:, j, 0:CHUNK],
                            lhsT=wmtiles[b, k][:, m * P:(m + 1) * P],
                            rhs=ytiles[b, k][:, n, :],
                            start=(i == 0),
                            stop=(i == KT - 1),
                        )

            # deferred per-engine closures so each engine's emission order
            # matches its runtime-ready order (engines execute in order)
            pend = {"dve_tail": [], "actA": [], "actB": []}

            def pop(q):
                if pend[q]:
                    pend[q].pop(0)()

            def emit_pw_fused(b, m):
                """12 matmuls into two 3-bank PSUM half-groups, then the
                first half's map max; queue the rest of the epilogue."""
                PZ = [pspw.tile([P, HCH, BANK], F32, name="PZ"),
                      pspw.tile([P, HCH, BANK], F32, name="PZ")]
                emit_pw_half_mms(PZ[0], b, m, 0)
                emit_pw_half_mms(PZ[1], b, m, 1)

                mab = st.tile([P, 2], F32, name="mab")
                # half-A map max can run as soon as the A matmuls finish
                nc.vector.tensor_reduce(
                    mab[:, 0:1], PZ[0][:, :, 0:CHUNK],
                    axis=mybir.AxisListType.XY, op=MAX)

                mask2 = st.tile([P, 1], F32, name="mask2")
                b2m = st.tile([P, 1], F32, name="b2m")

                def dve_tail():
                    nc.vector.tensor_reduce(
                        mab[:, 1:2], PZ[1][:, :, 0:CHUNK],
                        axis=mybir.AxisListType.XY, op=MAX)
                    m2 = st.tile([P, 1], F32, name="m2")
                    nc.vector.tensor_tensor(
                        m2, mab[:, 0:1], mab[:, 1:2], op=MAX)
                    # mask2 = (max(z) + b2 >= th)
                    nc.vector.tensor_scalar(
                        out=mask2, in0=m2, scalar1=b2sb[:, m:m + 1],
                        scalar2=PW_THRESH, op0=ADD, op1=mybir.AluOpType.is_ge)
                    nc.vector.tensor_tensor(
                        b2m, b2sb[:, m:m + 1], mask2, op=MULT)
                pend["dve_tail"].append(dve_tail)

                Z = zp.tile([P, NPIX], F32, name="Z")
                Z3 = Z.rearrange("p (c x) -> p c x", x=CHUNK)

                def act_drain(half):
                    # z = relu((z_nb + b2) * mask): scale=mask, bias=b2*mask
                    nc.scalar.activation(
                        out=Z3[:, half * HCH:(half + 1) * HCH, :],
                        in_=PZ[half][:, :, 0:CHUNK],
                        func=RELU, bias=b2m, scale=mask2)
                    nc.sync.dma_start(
                        out=zs_flat[b, m * P:(m + 1) * P,
                                    half * HPIX:(half + 1) * HPIX],
                        in_=Z[:, half * HPIX:(half + 1) * HPIX])
                pend["actA"].append(lambda: act_drain(0))
                pend["actB"].append(lambda: act_drain(1))

            def emit_pw_deferred(b, m, last=False):
                """No-filler m-tiles: drains run immediately (bias b2 - th)
                with accum side-sums; mask = (sum > 0), applied as a
                multiply. Work spread over ACT (drain A), DVE (drain B,
                mask, mult B) and GpSimd (mult A)."""
                PZ = [pspw.tile([P, HCH, BANK], F32, name="PZ"),
                      pspw.tile([P, HCH, BANK], F32, name="PZ")]
                Z = zp.tile([P, NPIX], F32, name="Z")
                Z3 = Z.rearrange("p (c x) -> p c x", x=CHUNK)
                asum = st.tile([P, 2], F32, name="asum")
                zeros3 = zerosb[:, 0:1].to_broadcast([P, HCH, CHUNK])
                emit_pw_half_mms(PZ[0], b, m, 0)
                nc.scalar.activation(
                    out=Z3[:, 0:HCH, :], in_=PZ[0][:, :, 0:CHUNK],
                    func=RELU, bias=b2thsb[:, m:m + 1], scale=1.0,
                    accum_out=asum[:, 0:1])
                emit_pw_half_mms(PZ[1], b, m, 1)
                nc.scalar.activation(
                    out=Z3[:, HCH:NCH, :], in_=PZ[1][:, :, 0:CHUNK],
                    func=RELU, bias=b2thsb[:, m:m + 1], scale=1.0,
                    accum_out=asum[:, 1:2])
                mask2 = st.tile([P, 1], F32, name="mask2")
                nc.vector.tensor_tensor(
                    mask2, asum[:, 0:1], asum[:, 1:2], op=ADD)
                nc.vector.tensor_scalar(
                    out=mask2, in0=mask2, scalar1=0.0, scalar2=None,
                    op0=mybir.AluOpType.is_gt)

                def mult_store(lo, hi, dma):
                    zpart = Z[:, lo:hi]
                    nc.vector.tensor_scalar(
                        out=zpart, in0=zpart, scalar1=mask2,
                        scalar2=None, op0=MULT)
                    dma(out=zs_flat[b, m * P:(m + 1) * P, lo:hi], in_=zpart)

                if last:
                    # quarter-granular masking and stores to shorten the
                    # final drain-out (gpsimd triggers avoided: its end
                    # drain would become the finish-barrier long pole)
                    mult_store(0, QPIX, nc.sync.dma_start)
                    mult_store(QPIX, 2 * QPIX, nc.scalar.dma_start)
                    mult_store(2 * QPIX, 3 * QPIX, nc.sync.dma_start)
                    mult_store(3 * QPIX, NPIX, nc.scalar.dma_start)
                else:
                    mult_store(0, HPIX, nc.sync.dma_start)
                    mult_store(HPIX, NPIX, nc.sync.dma_start)

            # ------------- program -------------
            # image 0 depthwise, solid (k1 first so its mask is ready when
            # the first pw m-tile starts)
            for k in KS:
                for n in range(NCH):
                    emit_dw_chunk(0, k, n)
                emit_dw_mask(0, k)

            # steady state: pw(b) m-tiles with dw(b+1) chunks as filler.
            # per-group emission order is tuned so each engine's in-order
            # queue matches readiness: [maxB+mask chain (m-1)] [fill f0]
            # [drain A(m-1)] [drain B(m-1)] [fill f1] [fill f2 (+ dw mask)]
            # [pw mms (m)] [maxA(m)]
            FILLS = [3, 3, 3, 3]
            for b in range(BPC - 1):
                chunks = [(k, n) for k in KS for n in range(NCH)]
                ci = 0
                for m in range(MT):
                    pop("dve_tail")
                    for j in range(FILLS[m]):
                        k, n = chunks[ci]
                        ci += 1
                        emit_dw_chunk(b + 1, k, n)
                        if j == 0:
                            pop("actA")
                        if j == 1:
                            pop("actB")
                        if n == NCH - 1:
                            emit_dw_mask(b + 1, k)
                    if b == BPC - 2 and m == MT - 1:
                        # last fused slot: go deferred so the b3 phase is
                        # not gated on this m-tile's mask chain
                        pop("dve_tail")
                        pop("actA")
                        pop("actB")
                        emit_pw_deferred(b, m)
                    else:
                        emit_pw_fused(b, m)
            # last image: deferred-mask epilogue throughout
            for m in range(MT):
                emit_pw_deferred(BPC - 1, m, last=(m == MT - 1))
    nc.compile()
    return nc


def _prep_params(dw_w, dw_b, dw_gamma, dw_beta, dw_mean, dw_var,
                 pw_w, pw_b, pw_gamma, pw_beta, pw_mean, pw_var):
    dw_scale = dw_gamma / np.sqrt(dw_var + EPS)
    b1 = dw_b * dw_scale + dw_beta - dw_mean * dw_scale          # (256,)
    w1 = dw_w[:, 0] * dw_scale[:, None, None]                    # (256,3,3)

    idx = np.arange(P)
    if DW_FP8:
        dwdiag = np.zeros((P, KT, 5, 2, P), np.float32)
        for k in range(KT):
            for p, (di, dj, _) in enumerate(DW_PAIRS):
                taps = [(di, dj)]
                flat = di * 3 + dj
                if flat + 1 < 9:
                    taps.append(((flat + 1) // 3, (flat + 1) % 3))
                for j, (ti, tj) in enumerate(taps):
                    dwdiag[idx, k, p, j, idx] = w1[k * P:(k + 1) * P, ti, tj]
        dwdiag = np.ascontiguousarray(dwdiag).astype(ml_dtypes.float8_e4m3)
    else:
        dwdiag = np.zeros((P, KT, 9, P), np.float32)
        for k in range(KT):
            for t in range(9):
                dwdiag[idx, k, t, idx] = w1[k * P:(k + 1) * P, t // 3, t % 3]
        dwdiag = np.ascontiguousarray(dwdiag).astype(ml_dtypes.bfloat16)

    pw_scale = pw_gamma / np.sqrt(pw_var + EPS)
    b2 = pw_b * pw_scale + pw_beta - pw_mean * pw_scale          # (512,)
    w2 = pw_w * pw_scale[:, None]                                # (512,256)
    # w2t[ck, k, o] = w2[o, k*128+ck]
    w2t = np.ascontiguousarray(
        w2.T.reshape(KT, P, COUT).transpose(1, 0, 2)).astype(np.float32)
    b1t = np.ascontiguousarray(b1.reshape(KT, P).T).astype(np.float32)
    b2t = np.ascontiguousarray(b2.reshape(MT, P).T).astype(np.float32)
    return (dwdiag, w2t.astype(ml_dtypes.bfloat16), b1t, b2t)


def kernel(x, dw_w, dw_b, dw_gamma, dw_beta, dw_mean, dw_var,
           pw_w, pw_b, pw_gamma, pw_beta, pw_mean, pw_var):
    global _cached_nc
    x = np.ascontiguousarray(np.asarray(x, np.float32))
    args = [np.asarray(a, np.float32) for a in
            (dw_w, dw_b, dw_gamma, dw_beta, dw_mean, dw_var,
             pw_w, pw_b, pw_gamma, pw_beta, pw_mean, pw_var)]
    dwdiag, w2t, b1t, b2t = _prep_params(*args)
    xdt = ml_dtypes.float8_e4m3 if DW_FP8 else ml_dtypes.bfloat16
    x16 = x.astype(xdt)

    if _cached_nc is None:
        _cached_nc = _build_program()
    nc = _cached_nc

    in_maps = []
    for c in range(NCORES):
        in_maps.append({
            "xs": np.ascontiguousarray(x16[c * BPC:(c + 1) * BPC]),
            "dwdiag": dwdiag,
            "w2t": w2t,
            "b1t": b1t,
            "b2t": b2t,
        })
    res = run_bass_kernel_spmd(nc, in_maps, core_ids=list(range(NCORES)))
    out = np.concatenate([res.results[c]["zs"] for c in range(NCORES)], axis=0)
    return out
